# revision 88
# baseline (speedup 1.0000x reference)
"""Decode-step KV-cache attention kernel for 8 Trainium2 NeuronCores.

Tensor-parallel over heads (2 heads per core, all 32 batch rows on every
core); per-core differences live in host-sliced inputs.  All bulk data is
bf16 (tolerance is 2e-2; bf16 keeps norm-relative error ~1e-3), which
halves HBM traffic — the binding constraint for this memory-regime problem
— and runs the PE at 1 cycle/row instead of fp32's 4.

Per-core pipeline (rows host-sorted by sequence length, descending;
adjacent small rows batched into shared 16 KB/partition DMA groups to
amortize the ~1.4 us fixed cost per DMA dispatch; K streams on the
scalar hardware-DGE queue, V on gpsimd, tiny traffic on sync — two big
queues saturate the ~310 GB/s DMA fabric):
  1. QKV projection: x^T tiles (PE transposes) x bf16 W_in -> PSUM.
     q and k_new are produced directly in [d, row] orientation by using
     the W_in columns as the matmul output partition dim.
  2. Scores, per (row, head, tile): one matmul with the host-TRANSPOSED
     K tile [d, tokens] as the stationary weights and the q column as
     the single moving vector -> scores [128 tokens, 1] in PSUM.  The
     new token's k/v are copied into the K/V tiles at position L-1
     beforehand (v via SBUF->SBUF DMA: engines cannot move data across
     partitions), so there is no separate new-token path.
  3. Exp, per row: one activation over [128, 2, n_tiles] (both heads).
     Host-zeroed pad tokens come out as exp(0)=1 and are removed from
     the denominator by a host-computed correction constant.
  4. PV, per (row, tile): one matmul with the 2 probability columns as
     weights and the V tile [tokens, 256ch] streaming -> [2, 256] PSUM;
     a second tiny matmul against a ones vector accumulates the softmax
     denominators with the same weights.  PV trails scores by 3 rows so
     the in-order PE stream never blocks on the insert/exp chain.
  5. Per row: subtract pad correction, reciprocal, normalize on the way
     out of PSUM, PE-transpose into [d, rowhead] for the out-projection.
  6. Out-project with bf16 W_out, PSUM->SBUF->HBM pipelined in 512-col
     chunks; host sums the 8 per-core partials and adds b_out.
"""

import math
import sys

import numpy as np
import ml_dtypes

sys.path.insert(0, "/opt/trn_rl_repo")

import concourse.bass as bass  # noqa: E402
import concourse.tile as tile  # noqa: E402
from concourse import bacc, mybir  # noqa: E402
from concourse.bass_utils import run_bass_kernel_spmd  # noqa: E402
from concourse.masks import make_identity  # noqa: E402

B, S_MAX, H, D = 32, 2048, 16, 128
E = H * D  # 2048
N_CORES = 8
H_LOC = H // N_CORES  # 2 heads per core
CLOC = H_LOC * D  # 256
ET = E // 128  # 16 contraction tiles for the in-projection

F32 = mybir.dt.float32
BF16 = mybir.dt.bfloat16
NPBF = ml_dtypes.bfloat16
EXP = mybir.ActivationFunctionType.Exp

_build_cache: dict = {}
LAST_RESULT = None  # last BassKernelResults, for test harness introspection


def make_groups(nts):
    """Group adjacent rows so several small rows share one DMA + SBUF tile.
    Group size n at max-tile-count m keeps n*m <= 32 (16 KB/partition); a
    strict pad budget keeps the padding bytes negligible."""
    groups = []
    j = 0
    while j < B:
        m = nts[j]
        n = 1
        pad = 0
        while j + n < B and n < 16:
            m2 = max(m, nts[j + n])
            pad2 = pad + (m2 - m) * n + (m2 - nts[j + n])
            if (n + 1) * m2 > 32 or pad2 > 2:
                break
            m, pad = m2, pad2
            n += 1
        groups.append(list(range(j, j + n)))
        j += n
    return groups


def _build(Ls: tuple, obs: tuple) -> bass.Bass:
    """Per-core Bass program. Ls = sorted (descending) seq lengths;
    obs[j] = original batch index of sorted row j."""
    nts = [(l + 127) // 128 for l in Ls]
    groups = make_groups(nts)
    g_of = {}
    r_of = {}
    k_rof = {}  # row's K base offset within its group tile (exact lengths)
    for gi, grp in enumerate(groups):
        off = 0
        for r, j in enumerate(grp):
            g_of[j], r_of[j] = gi, r
            k_rof[j] = off
            off += 2 * Ls[j]
    ntg = [max(nts[j] for j in grp) for grp in groups]
    k_goff = []  # element offsets of each group's K/V block in packed bufs
    v_goff = []
    k_gsz = []
    ko = vo = 0
    for gi, grp in enumerate(groups):
        k_goff.append(ko)
        v_goff.append(vo)
        k_gsz.append(sum(2 * Ls[j] for j in grp))
        ko += 128 * k_gsz[gi]
        vo += 128 * len(grp) * ntg[gi] * 256

    nc = bacc.Bacc("TRN2")
    x_d = nc.dram_tensor("x", [B, E], BF16, kind="ExternalInput")
    win_d = nc.dram_tensor("win", [128, ET * 3 * CLOC], BF16, kind="ExternalInput")
    bin_d = nc.dram_tensor("bin", [1, 3 * CLOC], BF16, kind="ExternalInput")
    wout_d = nc.dram_tensor("wout", [128, H_LOC * E], BF16, kind="ExternalInput")
    kc_d = nc.dram_tensor("kc", [ko], BF16, kind="ExternalInput")
    vc_d = nc.dram_tensor("vc", [vo], BF16, kind="ExternalInput")
    out_d = nc.dram_tensor("out", [B, E], F32, kind="ExternalOutput")

    kc_base = kc_d[:]
    vc_base = vc_d[:]
    inv_sqrt_d = 1.0 / math.sqrt(D)

    with tile.TileContext(nc) as tc:
        with tc.tile_pool(name="const", bufs=1) as const:
            I64 = const.tile([64, 64], BF16)
            make_identity(nc, I64)
            I32 = I64[0:32, 0:32]
            ones_1x32 = const.tile([1, 32], BF16)
            nc.vector.memset(ones_1x32, 1.0)
            ones_128 = const.tile([128, 1], BF16)
            nc.vector.memset(ones_128, 1.0)

            x_sb = const.tile([B, E], BF16)
            nc.scalar.dma_start(out=x_sb, in_=x_d[:])
            win_sb = const.tile([128, ET, 3 * CLOC], BF16)
            for a in range(4):
                nc.scalar.dma_start(
                    out=win_sb[:, 4 * a : 4 * a + 4, :],
                    in_=win_d[:, a * 4 * 3 * CLOC : (a + 1) * 4 * 3 * CLOC],
                )
            bin_sb = const.tile([1, 3 * CLOC], BF16)
            nc.scalar.dma_start(out=bin_sb, in_=bin_d[:])
            wout_sb = const.tile([128, H_LOC, E], BF16)
            nc.sync.dma_start(out=wout_sb, in_=wout_d[:])

            xT_sb = const.tile([128, ET, B], BF16)
            v_new_sb = const.tile([B, CLOC], BF16)
            qT_sb = const.tile([128, H_LOC, B], BF16)
            k_newT_sb = const.tile([128, H_LOC, B], BF16)
            aT_sb = const.tile([128, H_LOC, 2 * B], BF16)
            out_sb = const.tile([B, E], F32)

            # ---------------- phase 1: fused QKV projection ----------------
            # q and k_new are produced directly in [d, row] orientation by
            # using the W_in columns as the output partition dim.
            with tc.tile_pool(name="ph1ps", bufs=2, space="PSUM") as ph1ps:
                with tc.tile_pool(name="qkvps", bufs=1, space="PSUM") as qkvps:
                    for t in range(ET):
                        xt_ps = ph1ps.tile([128, B], BF16)
                        nc.tensor.transpose(
                            xt_ps, x_sb[:, t * 128 : (t + 1) * 128], I32
                        )
                        nc.vector.tensor_copy(xT_sb[:, t, :], xt_ps)
                    v_ps = qkvps.tile([B, CLOC], F32, tag="v")
                    nc.tensor.matmul(
                        v_ps,
                        ones_1x32,
                        bin_sb[:, 2 * CLOC : 3 * CLOC],
                        start=True,
                        stop=False,
                    )
                    for t in range(ET):
                        nc.tensor.matmul(
                            v_ps,
                            xT_sb[:, t, :],
                            win_sb[:, t, 2 * CLOC : 3 * CLOC],
                            start=False,
                            stop=(t == ET - 1),
                        )
                    nc.vector.tensor_copy(v_new_sb, v_ps)
                    for h in range(H_LOC):
                        for base in (0, CLOC):
                            c0 = base + h * D
                            ps = qkvps.tile([128, B], F32, tag=f"qk{base}{h}")
                            nc.tensor.matmul(
                                ps,
                                bin_sb[:, c0 : c0 + D],
                                ones_1x32,
                                start=True,
                                stop=False,
                            )
                            for t in range(ET):
                                nc.tensor.matmul(
                                    ps,
                                    win_sb[:, t, c0 : c0 + D],
                                    xT_sb[:, t, :],
                                    start=False,
                                    stop=(t == ET - 1),
                                )
                            dst = qT_sb if base == 0 else k_newT_sb
                            nc.vector.tensor_copy(dst[:, h, :], ps)

            # ---------------- phase 2: scores -> exp -> PV ------------------
            with tc.tile_pool(name="scps", bufs=2, space="PSUM") as scps, \
                 tc.tile_pool(name="avps", bufs=2, space="PSUM") as avps, \
                 tc.tile_pool(name="dnp", bufs=2, space="PSUM") as dnp, \
                 tc.tile_pool(name="tps", bufs=2, space="PSUM") as tps, \
                 tc.tile_pool(name="ktp", bufs=4) as ktp, \
                 tc.tile_pool(name="vtp", bufs=4) as vtp, \
                 tc.tile_pool(name="prp", bufs=4) as prp, \
                 tc.tile_pool(name="arp", bufs=3) as arp:
                kts = [None] * len(groups)
                vts = [None] * len(groups)
                prs = [None] * B
                GELEMS = 8192  # 16 KB/partition group tiles

                def emit_scores(j):
                    L = Ls[j]
                    nt = nts[j]
                    gi, r = g_of[j], r_of[j]
                    n = len(groups[gi])
                    v_eng = nc.gpsimd
                    if r == 0:
                        kt = ktp.tile([128, GELEMS], BF16)
                        kts[gi] = kt
                        ksz = k_gsz[gi]
                        nc.scalar.dma_start(
                            out=kt[:, 0:ksz],
                            in_=bass.AP(
                                tensor=kc_base.tensor,
                                offset=k_goff[gi],
                                ap=[[ksz, 128], [1, ksz]],
                            ),
                        )
                        vt = vtp.tile([128, GELEMS], BF16)
                        vts[gi] = vt
                        vsz = n * ntg[gi] * 256
                        v_eng.dma_start(
                            out=vt[:, 0:vsz],
                            in_=bass.AP(
                                tensor=vc_base.tensor,
                                offset=v_goff[gi],
                                ap=[[vsz, 128], [1, vsz]],
                            ),
                        )
                    kt = kts[gi]
                    vt = vts[gi]
                    ko = k_rof[j]  # this row's K base within the tile
                    vo = r * ntg[gi] * 256
                    # fold the new token in at column L-1
                    col = L - 1
                    ob = obs[j]
                    for h in range(H_LOC):
                        nc.vector.tensor_copy(
                            kt[:, ko + h * L + col : ko + h * L + col + 1],
                            k_newT_sb[:, h, ob : ob + 1],
                        )
                    # cross-partition row insert must go through DMA; the
                    # sync queue is idle mid-kernel so head-blocking is free
                    p_new, t_new = col % 128, col // 128
                    nc.sync.dma_start(
                        out=vt[
                            p_new : p_new + 1,
                            vo + t_new * 256 : vo + (t_new + 1) * 256,
                        ],
                        in_=v_new_sb[ob : ob + 1, :],
                    )
                    sc = scps.tile([128, H_LOC, ET], F32)
                    pr = prp.tile([128, H_LOC, ET], BF16)
                    prs[j] = pr
                    rem = L - (nt - 1) * 128  # tokens in the last tile
                    for h in range(H_LOC):
                        for t in range(nt):
                            o = ko + h * L + t * 128
                            nc2 = 128 if t < nt - 1 else rem
                            nc.tensor.matmul(
                                sc[0:nc2, h, t : t + 1],
                                kt[:, o : o + nc2],
                                qT_sb[:, h, ob : ob + 1],
                                start=True,
                                stop=True,
                                skip_group_check=True,
                            )
                    if nt > 1:
                        nc.scalar.activation(
                            pr[:, :, 0 : nt - 1],
                            sc[:, :, 0 : nt - 1],
                            EXP,
                            scale=inv_sqrt_d,
                        )
                    nc.scalar.activation(
                        pr[0:rem, :, nt - 1 : nt],
                        sc[0:rem, :, nt - 1 : nt],
                        EXP,
                        scale=inv_sqrt_d,
                    )

                def emit_pv(j):
                    nt = nts[j]
                    gi, r = g_of[j], r_of[j]
                    vt = vts[gi]
                    vo = r * ntg[gi] * 256
                    pr = prs[j]
                    av = avps.tile([H_LOC, CLOC], F32)
                    den_ps = dnp.tile([H_LOC, 1], F32)
                    rem = Ls[j] - (nt - 1) * 128
                    for t in range(nt):
                        nc2 = 128 if t < nt - 1 else rem
                        nc.tensor.matmul(
                            av,
                            pr[0:nc2, :, t],
                            vt[0:nc2, vo + t * 256 : vo + (t + 1) * 256],
                            start=(t == 0),
                            stop=(t == nt - 1),
                            skip_group_check=True,
                        )
                        # denominator rides along: same weights, ones vector
                        nc.tensor.matmul(
                            den_ps,
                            pr[0:nc2, :, t],
                            ones_128[0:nc2, :],
                            start=(t == 0),
                            stop=(t == nt - 1),
                            skip_group_check=True,
                        )
                    ar = arp.tile([H_LOC, CLOC], BF16)
                    den2 = arp.tile([H_LOC, 2], F32, tag="dn")
                    nc.vector.reciprocal(den2[:, 1:2], den_ps)
                    nc.vector.tensor_scalar_mul(ar, av, den2[:, 1:2])
                    for h in range(H_LOC):
                        at_ps = tps.tile([128, H_LOC], BF16)
                        nc.tensor.transpose(
                            at_ps, ar[:, h * D : (h + 1) * D], I64[0:2, 0:2]
                        )
                        nc.vector.tensor_copy(
                            aT_sb[:, h, 2 * j : 2 * j + 2], at_ps
                        )

                LAG = 3
                for j in range(B):
                    emit_scores(j)
                    if j >= LAG:
                        emit_pv(j - LAG)
                for j in range(B - LAG, B):
                    emit_pv(j)

            # ---------------- phase 3: out-projection -----------------------
            with tc.tile_pool(name="outps", bufs=1, space="PSUM") as outps:
                    out_ps = outps.tile([B, E], F32)
                    for j4 in range(4):
                        for h in range(H_LOC):
                            base = aT_sb[:, h, :]
                            lhsT = bass.AP(
                                tensor=base.tensor,
                                offset=base.offset + h,
                                ap=[base.ap[0], [2, B]],
                            )
                            nc.tensor.matmul(
                                out_ps[:, j4 * 512 : (j4 + 1) * 512],
                                lhsT,
                                wout_sb[:, h, j4 * 512 : (j4 + 1) * 512],
                                start=(h == 0),
                                stop=(h == H_LOC - 1),
                            )
                        s = slice(j4 * 512, (j4 + 1) * 512)
                        nc.vector.tensor_copy(out_sb[:, s], out_ps[:, s])
                        nc.sync.dma_start(out=out_d[:, s], in_=out_sb[:, s])
    nc.compile()
    return nc


def kernel(x, k_cache, v_cache, W_in, b_in, W_out, b_out, input_pos):
    global LAST_RESULT
    x = np.asarray(x)
    k_cache = np.asarray(k_cache)
    v_cache = np.asarray(v_cache)
    W_in = np.asarray(W_in, dtype=np.float32)
    b_in = np.asarray(b_in, dtype=np.float32)
    W_out = np.asarray(W_out, dtype=np.float32)
    b_out = np.asarray(b_out, dtype=np.float32)
    pos = np.asarray(input_pos).astype(np.int64)

    order = sorted(range(B), key=lambda b: -int(pos[b]))
    Ls = tuple(int(pos[b]) for b in order)
    nts = [(l + 127) // 128 for l in Ls]
    groups = make_groups(nts)

    key = (Ls, tuple(order))
    if key not in _build_cache:
        _build_cache[key] = _build(Ls, tuple(order))
    nc = _build_cache[key]

    x2 = np.ascontiguousarray(x.reshape(B, E)).astype(NPBF)
    kc_bf = k_cache.astype(NPBF)
    vc_bf = v_cache.astype(NPBF)

    in_maps = []
    for i in range(N_CORES):
        c0 = i * CLOC
        win_i = np.concatenate(
            [
                W_in[:, c0 : c0 + CLOC],
                W_in[:, E + c0 : E + c0 + CLOC],
                W_in[:, 2 * E + c0 : 2 * E + c0 + CLOC],
            ],
            axis=1,
        )  # [2048, 768]
        win_i = np.ascontiguousarray(
            win_i.reshape(ET, 128, 3 * CLOC).transpose(1, 0, 2).reshape(128, -1)
        ).astype(NPBF)
        bin_i = np.concatenate(
            [
                b_in[c0 : c0 + CLOC],
                b_in[E + c0 : E + c0 + CLOC],
                b_in[2 * E + c0 : 2 * E + c0 + CLOC],
            ]
        )[None, :].astype(NPBF)
        wout_i = np.ascontiguousarray(
            W_out[c0 : c0 + CLOC, :].reshape(H_LOC, 128, E)
            .transpose(1, 0, 2)
            .reshape(128, -1)
        ).astype(NPBF)
        h0 = i * H_LOC
        k_h = kc_bf[:, :, h0 : h0 + H_LOC, :]  # [B, S, 2, 128]
        v_h = vc_bf[:, :, h0 : h0 + H_LOC, :]
        k_blocks = []
        v_blocks = []
        for grp in groups:
            spg = max(nts[j] for j in grp) * 128
            ksz = sum(2 * Ls[j] for j in grp)
            kg = np.zeros((128, ksz), dtype=NPBF)
            vg = np.zeros((128, len(grp), spg // 128, CLOC), dtype=NPBF)
            off = 0
            for r, j in enumerate(grp):
                ob = order[j]
                L = Ls[j]
                if L > 1:
                    kb = np.zeros((128, H_LOC, L), dtype=NPBF)
                    kb[:, :, : L - 1] = k_h[ob, : L - 1].transpose(2, 1, 0)
                    kg[:, off : off + 2 * L] = kb.reshape(128, 2 * L)
                    vb = np.zeros((spg, CLOC), dtype=NPBF)
                    vb[: L - 1] = v_h[ob, : L - 1].reshape(L - 1, CLOC)
                    vg[:, r] = vb.reshape(spg // 128, 128, CLOC).transpose(
                        1, 0, 2
                    )
                off += 2 * L
            k_blocks.append(kg.ravel())
            v_blocks.append(vg.ravel())
        kc_i = np.concatenate(k_blocks)
        vc_i = np.concatenate(v_blocks)
        in_maps.append(
            {
                "x": x2,
                "win": win_i,
                "bin": bin_i,
                "wout": wout_i,
                "kc": kc_i,
                "vc": vc_i,
            }
        )

    res = run_bass_kernel_spmd(nc, in_maps, core_ids=list(range(N_CORES)))
    LAST_RESULT = res
    acc = np.zeros((B, E), dtype=np.float64)
    for r in res.results:
        acc += r["out"].astype(np.float64)
    acc += b_out.astype(np.float64)
    out = np.zeros((B, E), dtype=np.float32)
    out[np.array(order)] = acc.astype(np.float32)
    return out.reshape(B, 1, E)


# revision 92
# speedup vs baseline: 1.0155x; 1.0155x over previous
"""Decode-step KV-cache attention kernel for 8 Trainium2 NeuronCores.

Tensor-parallel over heads (2 heads per core, all 32 batch rows on every
core); per-core differences live in host-sliced inputs.  All bulk data is
bf16 (tolerance is 2e-2; bf16 keeps norm-relative error ~1e-3), which
halves HBM traffic — the binding constraint for this memory-regime problem
— and runs the PE at 1 cycle/row instead of fp32's 4.

Per-core pipeline (rows host-sorted by sequence length, descending;
adjacent small rows batched into shared 16 KB/partition DMA groups to
amortize the ~1.4 us fixed cost per DMA dispatch; K streams on the
scalar hardware-DGE queue, V on gpsimd, tiny traffic on sync — two big
queues saturate the ~310 GB/s DMA fabric):
  1. QKV projection: x^T tiles (PE transposes) x bf16 W_in -> PSUM.
     q and k_new are produced directly in [d, row] orientation by using
     the W_in columns as the matmul output partition dim.
  2. Scores, per (row, head, tile): one matmul with the host-TRANSPOSED
     K tile [d, tokens] as the stationary weights and the q column as
     the single moving vector -> scores [128 tokens, 1] in PSUM.  The
     new token's k/v are copied into the K/V tiles at position L-1
     beforehand (v via SBUF->SBUF DMA: engines cannot move data across
     partitions), so there is no separate new-token path.
  3. Exp, per row: one activation over [128, 2, n_tiles] (both heads).
     Host-zeroed pad tokens come out as exp(0)=1 and are removed from
     the denominator by a host-computed correction constant.
  4. PV, per (row, tile): one matmul with the 2 probability columns as
     weights and the V tile [tokens, 256ch] streaming -> [2, 256] PSUM;
     a second tiny matmul against a ones vector accumulates the softmax
     denominators with the same weights.  PV trails scores by 3 rows so
     the in-order PE stream never blocks on the insert/exp chain.
  5. Per row: subtract pad correction, reciprocal, normalize on the way
     out of PSUM, PE-transpose into [d, rowhead] for the out-projection.
  6. Out-project with bf16 W_out, PSUM->SBUF->HBM pipelined in 512-col
     chunks; host sums the 8 per-core partials and adds b_out.
"""

import math
import sys

import numpy as np
import ml_dtypes

sys.path.insert(0, "/opt/trn_rl_repo")

import concourse.bass as bass  # noqa: E402
import concourse.tile as tile  # noqa: E402
from concourse import bacc, mybir  # noqa: E402
from concourse.bass_utils import run_bass_kernel_spmd  # noqa: E402
from concourse.masks import make_identity  # noqa: E402

B, S_MAX, H, D = 32, 2048, 16, 128
E = H * D  # 2048
N_CORES = 8
H_LOC = H // N_CORES  # 2 heads per core
CLOC = H_LOC * D  # 256
ET = E // 128  # 16 contraction tiles for the in-projection

F32 = mybir.dt.float32
BF16 = mybir.dt.bfloat16
NPBF = ml_dtypes.bfloat16
EXP = mybir.ActivationFunctionType.Exp

_build_cache: dict = {}
LAST_RESULT = None  # last BassKernelResults, for test harness introspection


def make_groups(nts):
    """Group adjacent rows so several small rows share one DMA + SBUF tile.
    Group size n at max-tile-count m keeps n*m <= 32 (16 KB/partition); a
    strict pad budget keeps the padding bytes negligible."""
    groups = []
    j = 0
    while j < B:
        m = nts[j]
        n = 1
        pad = 0
        while j + n < B and n < 16:
            m2 = max(m, nts[j + n])
            pad2 = pad + (m2 - m) * n + (m2 - nts[j + n])
            if (n + 1) * m2 > 32 or pad2 > 2:
                break
            m, pad = m2, pad2
            n += 1
        groups.append(list(range(j, j + n)))
        j += n
    return groups


def _build(Ls: tuple, obs: tuple) -> bass.Bass:
    """Per-core Bass program. Ls = sorted (descending) seq lengths;
    obs[j] = original batch index of sorted row j."""
    nts = [(l + 127) // 128 for l in Ls]
    groups = make_groups(nts)
    g_of = {}
    r_of = {}
    k_rof = {}  # row's K base offset within its group tile (exact lengths)
    for gi, grp in enumerate(groups):
        off = 0
        for r, j in enumerate(grp):
            g_of[j], r_of[j] = gi, r
            k_rof[j] = off
            off += 2 * Ls[j]
    ntg = [max(nts[j] for j in grp) for grp in groups]
    k_goff = []  # element offsets of each group's K/V block in packed bufs
    v_goff = []
    k_gsz = []
    ko = vo = 0
    for gi, grp in enumerate(groups):
        k_goff.append(ko)
        v_goff.append(vo)
        k_gsz.append(sum(2 * Ls[j] for j in grp))
        ko += 128 * k_gsz[gi]
        vo += 128 * len(grp) * ntg[gi] * 256

    nc = bacc.Bacc("TRN2")
    x_d = nc.dram_tensor("x", [B, E], BF16, kind="ExternalInput")
    win_d = nc.dram_tensor("win", [128, ET * 3 * CLOC], BF16, kind="ExternalInput")
    bin_d = nc.dram_tensor("bin", [1, 3 * CLOC], BF16, kind="ExternalInput")
    wout_d = nc.dram_tensor("wout", [128, H_LOC * E], BF16, kind="ExternalInput")
    kc_d = nc.dram_tensor("kc", [ko], BF16, kind="ExternalInput")
    vc_d = nc.dram_tensor("vc", [vo], BF16, kind="ExternalInput")
    out_d = nc.dram_tensor("out", [B, E], F32, kind="ExternalOutput")

    kc_base = kc_d[:]
    vc_base = vc_d[:]
    inv_sqrt_d = 1.0 / math.sqrt(D)

    with tile.TileContext(nc) as tc:
        with tc.tile_pool(name="const", bufs=1) as const:
            I64 = const.tile([64, 64], BF16)
            make_identity(nc, I64)
            I32 = I64[0:32, 0:32]
            ones_1x32 = const.tile([1, 32], BF16)
            nc.vector.memset(ones_1x32, 1.0)
            ones_128 = const.tile([128, 1], BF16)
            nc.vector.memset(ones_128, 1.0)

            x_sb = const.tile([B, E], BF16)
            nc.scalar.dma_start(out=x_sb, in_=x_d[:])
            win_sb = const.tile([128, ET, 3 * CLOC], BF16)
            for a in range(4):
                nc.scalar.dma_start(
                    out=win_sb[:, 4 * a : 4 * a + 4, :],
                    in_=win_d[:, a * 4 * 3 * CLOC : (a + 1) * 4 * 3 * CLOC],
                )
            bin_sb = const.tile([1, 3 * CLOC], BF16)
            nc.scalar.dma_start(out=bin_sb, in_=bin_d[:])
            wout_sb = const.tile([128, H_LOC, E], BF16)
            nc.sync.dma_start(out=wout_sb, in_=wout_d[:])

            xT_sb = const.tile([128, ET, B], BF16)
            v_new_sb = const.tile([B, CLOC], BF16)
            qT_sb = const.tile([128, H_LOC, B], BF16)
            k_newT_sb = const.tile([128, H_LOC, B], BF16)
            aT_sb = const.tile([128, H_LOC, 2 * B], BF16)
            out_sb = const.tile([B, E], F32)

            # ---------------- phase 1: fused QKV projection ----------------
            # q and k_new are produced directly in [d, row] orientation by
            # using the W_in columns as the output partition dim.
            with tc.tile_pool(name="ph1ps", bufs=2, space="PSUM") as ph1ps:
                with tc.tile_pool(name="qkvps", bufs=1, space="PSUM") as qkvps:
                    for t in range(ET):
                        xt_ps = ph1ps.tile([128, B], BF16)
                        nc.tensor.transpose(
                            xt_ps, x_sb[:, t * 128 : (t + 1) * 128], I32
                        )
                        nc.vector.tensor_copy(xT_sb[:, t, :], xt_ps)
                    v_ps = qkvps.tile([B, CLOC], F32, tag="v")
                    nc.tensor.matmul(
                        v_ps,
                        ones_1x32,
                        bin_sb[:, 2 * CLOC : 3 * CLOC],
                        start=True,
                        stop=False,
                    )
                    for t in range(ET):
                        nc.tensor.matmul(
                            v_ps,
                            xT_sb[:, t, :],
                            win_sb[:, t, 2 * CLOC : 3 * CLOC],
                            start=False,
                            stop=(t == ET - 1),
                        )
                    nc.vector.tensor_copy(v_new_sb, v_ps)
                    for h in range(H_LOC):
                        for base in (0, CLOC):
                            c0 = base + h * D
                            ps = qkvps.tile([128, B], F32, tag=f"qk{base}{h}")
                            nc.tensor.matmul(
                                ps,
                                bin_sb[:, c0 : c0 + D],
                                ones_1x32,
                                start=True,
                                stop=False,
                            )
                            for t in range(ET):
                                nc.tensor.matmul(
                                    ps,
                                    win_sb[:, t, c0 : c0 + D],
                                    xT_sb[:, t, :],
                                    start=False,
                                    stop=(t == ET - 1),
                                )
                            dst = qT_sb if base == 0 else k_newT_sb
                            nc.vector.tensor_copy(dst[:, h, :], ps)

            # ---------------- phase 2: scores -> exp -> PV ------------------
            with tc.tile_pool(name="scps", bufs=2, space="PSUM") as scps, \
                 tc.tile_pool(name="avps", bufs=2, space="PSUM") as avps, \
                 tc.tile_pool(name="dnp", bufs=2, space="PSUM") as dnp, \
                 tc.tile_pool(name="tps", bufs=2, space="PSUM") as tps, \
                 tc.tile_pool(name="ktp", bufs=4) as ktp, \
                 tc.tile_pool(name="vtp", bufs=4) as vtp, \
                 tc.tile_pool(name="prp", bufs=4) as prp, \
                 tc.tile_pool(name="arp", bufs=3) as arp:
                kts = [None] * len(groups)
                vts = [None] * len(groups)
                prs = [None] * B
                GELEMS = 8192  # 16 KB/partition group tiles

                def emit_scores(j):
                    L = Ls[j]
                    nt = nts[j]
                    gi, r = g_of[j], r_of[j]
                    n = len(groups[gi])
                    v_eng = nc.gpsimd
                    if r == 0:
                        kt = ktp.tile([128, GELEMS], BF16)
                        kts[gi] = kt
                        ksz = k_gsz[gi]
                        nc.scalar.dma_start(
                            out=kt[:, 0:ksz],
                            in_=bass.AP(
                                tensor=kc_base.tensor,
                                offset=k_goff[gi],
                                ap=[[ksz, 128], [1, ksz]],
                            ),
                        )
                        vt = vtp.tile([128, GELEMS], BF16)
                        vts[gi] = vt
                        vsz = n * ntg[gi] * 256
                        v_eng.dma_start(
                            out=vt[:, 0:vsz],
                            in_=bass.AP(
                                tensor=vc_base.tensor,
                                offset=v_goff[gi],
                                ap=[[vsz, 128], [1, vsz]],
                            ),
                        )
                    kt = kts[gi]
                    vt = vts[gi]
                    ko = k_rof[j]  # this row's K base within the tile
                    vo = r * ntg[gi] * 256
                    # fold the new token in at column L-1
                    col = L - 1
                    ob = obs[j]
                    for h in range(H_LOC):
                        nc.vector.tensor_copy(
                            kt[:, ko + h * L + col : ko + h * L + col + 1],
                            k_newT_sb[:, h, ob : ob + 1],
                        )
                    # cross-partition row insert must go through DMA; the
                    # sync queue is idle mid-kernel so head-blocking is free
                    p_new, t_new = col % 128, col // 128
                    nc.sync.dma_start(
                        out=vt[
                            p_new : p_new + 1,
                            vo + t_new * 256 : vo + (t_new + 1) * 256,
                        ],
                        in_=v_new_sb[ob : ob + 1, :],
                    )
                    sc = scps.tile([128, H_LOC, ET], F32)
                    pr = prp.tile([128, H_LOC, ET], BF16)
                    prs[j] = pr
                    rem = L - (nt - 1) * 128  # tokens in the last tile
                    for h in range(H_LOC):
                        for t in range(nt):
                            o = ko + h * L + t * 128
                            nc2 = 128 if t < nt - 1 else rem
                            nc.tensor.matmul(
                                sc[0:nc2, h, t : t + 1],
                                kt[:, o : o + nc2],
                                qT_sb[:, h, ob : ob + 1],
                                start=True,
                                stop=True,
                                skip_group_check=True,
                            )
                    if nt > 1:
                        nc.scalar.activation(
                            pr[:, :, 0 : nt - 1],
                            sc[:, :, 0 : nt - 1],
                            EXP,
                            scale=inv_sqrt_d,
                        )
                    nc.scalar.activation(
                        pr[0:rem, :, nt - 1 : nt],
                        sc[0:rem, :, nt - 1 : nt],
                        EXP,
                        scale=inv_sqrt_d,
                    )

                def emit_pv(j):
                    nt = nts[j]
                    gi, r = g_of[j], r_of[j]
                    vt = vts[gi]
                    vo = r * ntg[gi] * 256
                    pr = prs[j]
                    av = avps.tile([H_LOC, CLOC], F32)
                    den_ps = dnp.tile([H_LOC, 1], F32)
                    rem = Ls[j] - (nt - 1) * 128
                    for t in range(nt):
                        nc2 = 128 if t < nt - 1 else rem
                        nc.tensor.matmul(
                            av,
                            pr[0:nc2, :, t],
                            vt[0:nc2, vo + t * 256 : vo + (t + 1) * 256],
                            start=(t == 0),
                            stop=(t == nt - 1),
                            skip_group_check=True,
                        )
                        # denominator rides along: same weights, ones vector
                        nc.tensor.matmul(
                            den_ps,
                            pr[0:nc2, :, t],
                            ones_128[0:nc2, :],
                            start=(t == 0),
                            stop=(t == nt - 1),
                            skip_group_check=True,
                        )
                    ar = arp.tile([H_LOC, CLOC], BF16)
                    den2 = arp.tile([H_LOC, 2], F32, tag="dn")
                    nc.vector.reciprocal(den2[:, 1:2], den_ps)
                    nc.vector.tensor_scalar_mul(ar, av, den2[:, 1:2])
                    for h in range(H_LOC):
                        at_ps = tps.tile([128, H_LOC], BF16)
                        nc.tensor.transpose(
                            at_ps, ar[:, h * D : (h + 1) * D], I64[0:2, 0:2]
                        )
                        nc.vector.tensor_copy(
                            aT_sb[:, h, 2 * j : 2 * j + 2], at_ps
                        )

                LAG = 3
                for j in range(B):
                    emit_scores(j)
                    if j >= LAG:
                        emit_pv(j - LAG)
                for j in range(B - LAG, B):
                    emit_pv(j)

            # ---------------- phase 3: out-projection -----------------------
            with tc.tile_pool(name="outps", bufs=1, space="PSUM") as outps:
                    out_ps = outps.tile([B, E], F32)
                    for j4 in range(4):
                        for h in range(H_LOC):
                            base = aT_sb[:, h, :]
                            lhsT = bass.AP(
                                tensor=base.tensor,
                                offset=base.offset + h,
                                ap=[base.ap[0], [2, B]],
                            )
                            nc.tensor.matmul(
                                out_ps[:, j4 * 512 : (j4 + 1) * 512],
                                lhsT,
                                wout_sb[:, h, j4 * 512 : (j4 + 1) * 512],
                                start=(h == 0),
                                stop=(h == H_LOC - 1),
                            )
                        s = slice(j4 * 512, (j4 + 1) * 512)
                        nc.vector.tensor_copy(out_sb[:, s], out_ps[:, s])
                        nc.sync.dma_start(out=out_d[:, s], in_=out_sb[:, s])
    nc.compile()
    return nc


def kernel(x, k_cache, v_cache, W_in, b_in, W_out, b_out, input_pos):
    global LAST_RESULT
    x = np.asarray(x)
    k_cache = np.asarray(k_cache)
    v_cache = np.asarray(v_cache)
    W_in = np.asarray(W_in, dtype=np.float32)
    b_in = np.asarray(b_in, dtype=np.float32)
    W_out = np.asarray(W_out, dtype=np.float32)
    b_out = np.asarray(b_out, dtype=np.float32)
    pos = np.asarray(input_pos).astype(np.int64)

    order = sorted(range(B), key=lambda b: -int(pos[b]))
    Ls = tuple(int(pos[b]) for b in order)
    nts = [(l + 127) // 128 for l in Ls]
    groups = make_groups(nts)

    key = (Ls, tuple(order))
    if key not in _build_cache:
        _build_cache[key] = _build(Ls, tuple(order))
    nc = _build_cache[key]

    x2 = np.ascontiguousarray(x.reshape(B, E)).astype(NPBF)
    kc_bf = k_cache.astype(NPBF)
    vc_bf = v_cache.astype(NPBF)

    in_maps = []
    for i in range(N_CORES):
        c0 = i * CLOC
        win_i = np.concatenate(
            [
                W_in[:, c0 : c0 + CLOC],
                W_in[:, E + c0 : E + c0 + CLOC],
                W_in[:, 2 * E + c0 : 2 * E + c0 + CLOC],
            ],
            axis=1,
        )  # [2048, 768]
        win_i = np.ascontiguousarray(
            win_i.reshape(ET, 128, 3 * CLOC).transpose(1, 0, 2).reshape(128, -1)
        ).astype(NPBF)
        bin_i = np.concatenate(
            [
                b_in[c0 : c0 + CLOC],
                b_in[E + c0 : E + c0 + CLOC],
                b_in[2 * E + c0 : 2 * E + c0 + CLOC],
            ]
        )[None, :].astype(NPBF)
        wout_i = np.ascontiguousarray(
            W_out[c0 : c0 + CLOC, :].reshape(H_LOC, 128, E)
            .transpose(1, 0, 2)
            .reshape(128, -1)
        ).astype(NPBF)
        h0 = i * H_LOC
        k_h = kc_bf[:, :, h0 : h0 + H_LOC, :]  # [B, S, 2, 128]
        v_h = vc_bf[:, :, h0 : h0 + H_LOC, :]
        k_blocks = []
        v_blocks = []
        for grp in groups:
            spg = max(nts[j] for j in grp) * 128
            ksz = sum(2 * Ls[j] for j in grp)
            kg = np.zeros((128, ksz), dtype=NPBF)
            vg = np.zeros((128, len(grp), spg // 128, CLOC), dtype=NPBF)
            off = 0
            for r, j in enumerate(grp):
                ob = order[j]
                L = Ls[j]
                if L > 1:
                    kb = np.zeros((128, H_LOC, L), dtype=NPBF)
                    kb[:, :, : L - 1] = k_h[ob, : L - 1].transpose(2, 1, 0)
                    kg[:, off : off + 2 * L] = kb.reshape(128, 2 * L)
                    vb = np.zeros((spg, CLOC), dtype=NPBF)
                    vb[: L - 1] = v_h[ob, : L - 1].reshape(L - 1, CLOC)
                    vg[:, r] = vb.reshape(spg // 128, 128, CLOC).transpose(
                        1, 0, 2
                    )
                off += 2 * L
            k_blocks.append(kg.ravel())
            v_blocks.append(vg.ravel())
        kc_i = np.concatenate(k_blocks)
        vc_i = np.concatenate(v_blocks)
        in_maps.append(
            {
                "x": x2,
                "win": win_i,
                "bin": bin_i,
                "wout": wout_i,
                "kc": kc_i,
                "vc": vc_i,
            }
        )

    res = run_bass_kernel_spmd(nc, in_maps, core_ids=list(range(N_CORES)))
    LAST_RESULT = res
    acc = np.zeros((B, E), dtype=np.float64)
    for r in res.results:
        acc += r["out"].astype(np.float64)
    acc += b_out.astype(np.float64)
    out = np.zeros((B, E), dtype=np.float32)
    out[np.array(order)] = acc.astype(np.float32)
    return out.reshape(B, 1, E)
